# revision 1
# baseline (speedup 1.0000x reference)
"""DiT block kernel for 8 trn2 NeuronCores.

Sharding: core c -> (batch b=c//2, query-token half h=c%2). Each core
computes the full block for its 512 query tokens (K/V compute for all
1024 tokens of its batch is replicated within the pair) -> zero
collectives. Activations are feature-major ([D on partitions, tokens on
free]); weights are used in natural [in, out] layout as matmul lhsT.
Matmuls run in fp16 (fp32 PSUM accumulation); LN stats, residuals and
softmax normalization run in fp32.
"""
import numpy as np

import concourse.bass as bass
import concourse.tile as tile
import concourse.mybir as mybir
from concourse.bass_utils import run_bass_kernel_spmd
from concourse.vector_clock import ScopedClock
from concourse.alu_op_type import AluOpType

dt = mybir.dt
AF = mybir.ActivationFunctionType

P = 128
B, NT, D, H = 4, 1024, 1024, 16
DH = D // H            # 64
DFF = 4 * D            # 4096
KC = D // P            # 8
LT = NT // 2           # 512 local query tokens
GATE = 0.1
EPS = 1e-5
EXP_SCALE = DH ** -0.5
EXP_BIAS = -3.0        # constant shift inside exp; cancels in softmax


class SplitDrainTileContext(tile.TileContext):
    """Tail drain in this walrus build holds few sync waits; spill the
    rest onto chained SP nops (runs before the sem-clear barrier, so
    semantics are preserved)."""

    MAX_TAIL_WAITS = 1

    def _drain_and_barrier(self, tick_clock, wait_clock):
        drain_inst = self.nc.sync.drain()
        wait_clock.add_sem_waits(
            drain_inst.ins, ScopedClock({None: tick_clock.global_clock})
        )
        si = drain_inst.ins.sync_info
        waits = list(si.on_wait) if si else []
        if len(waits) > self.MAX_TAIL_WAITS:
            drain_inst.ins.sync_info = mybir.SyncInfo(
                on_wait=waits[: self.MAX_TAIL_WAITS],
                on_update=list(si.on_update) if si else [],
            )
            rest = waits[self.MAX_TAIL_WAITS:]
            for i in range(0, len(rest), self.MAX_TAIL_WAITS):
                nop = self.nc.sync.nop()
                nop.ins.sync_info = mybir.SyncInfo(
                    on_wait=rest[i : i + self.MAX_TAIL_WAITS], on_update=[]
                )
        self.nc.all_engine_barrier()
        assert self.sems is not None
        popped = self.nc._tile_sem_poison_stack.pop()
        assert popped is self._sem_poison
        self.nc.clear_and_free_semaphores(list(self.sems.allocated().values()))
        self.nc.all_engine_barrier()


def _legalize_waits(nc, max_waits=1):
    """This walrus build accepts at most one sync wait per instruction.
    Move surplus waits onto same-engine NoOps inserted just before the
    offending instruction (engine FIFO order preserves semantics)."""
    fix = 0
    for bb in nc.main_func.blocks:
        insts = list(bb.instructions)
        out = []
        for inst in insts:
            si = inst.sync_info
            waits = list(si.on_wait) if si else []
            if len(waits) > max_waits:
                keep = waits[-max_waits:]
                for w in waits[:-max_waits]:
                    nop = mybir.InstNoOp(name=f"I-wfix{fix}")
                    fix += 1
                    nop.engine = inst.engine
                    nop.sync_info = mybir.SyncInfo(on_wait=[w], on_update=[])
                    out.append(nop)
                inst.sync_info = mybir.SyncInfo(
                    on_wait=keep, on_update=list(si.on_update) if si else [])
            out.append(inst)
        if len(out) != len(insts):
            bb.instructions = out
    return fix


def _build():
    nc = bass.Bass(target_bir_lowering=False, debug=False,
                   dynamic_dma_scratch_size=2048)
    f32, f16 = dt.float32, dt.float16

    xt = nc.dram_tensor("xt", [D, NT], f32, kind="ExternalInput")
    cond16 = nc.dram_tensor("cond16", [P, KC], f16, kind="ExternalInput")
    qkvw = nc.dram_tensor("qkvw", [D, 3 * D], f16, kind="ExternalInput")
    qkvbf = nc.dram_tensor("qkvbf", [P, 16], f32, kind="ExternalInput")
    bvrow = nc.dram_tensor("bvrow", [1, D], f16, kind="ExternalInput")
    projw = nc.dram_tensor("projw", [D, D], f16, kind="ExternalInput")
    projbrow = nc.dram_tensor("projbrow", [1, D], f16, kind="ExternalInput")
    fc1w = nc.dram_tensor("fc1w", [D, DFF], f16, kind="ExternalInput")
    fc1bf = nc.dram_tensor("fc1bf", [P, 32], f32, kind="ExternalInput")
    fc2w = nc.dram_tensor("fc2w", [D, DFF], f16, kind="ExternalInput")  # host-rearranged
    fc2brow = nc.dram_tensor("fc2brow", [1, D], f16, kind="ExternalInput")
    modw = nc.dram_tensor("modw", [12 * P, 4 * D], f16, kind="ExternalInput")  # host-rearranged
    modbf = nc.dram_tensor("modbf", [P, 6 * KC], f32, kind="ExternalInput")
    lnf = nc.dram_tensor("lnf", [P, 4 * KC], f32, kind="ExternalInput")
    outt = nc.dram_tensor("outt", [D, LT], f32, kind="ExternalOutput")

    with SplitDrainTileContext(nc) as tc:
        with tc.tile_pool(name="cp", bufs=1) as cp, \
             tc.tile_pool(name="ar", bufs=1) as ar, \
             tc.tile_pool(name="rot", bufs=4) as rot, \
             tc.tile_pool(name="psA", bufs=2, space="PSUM") as psA, \
             tc.tile_pool(name="psB", bufs=3, space="PSUM") as psB:

            def pp():    # [P, 512] f32 psum, 4 rotating banks
                return psA.tile([P, 512], f32, tag="pp", name="pp")

            def pbig():  # [P, 1024] f32 psum, 2 rotating 2-bank tiles
                return psB.tile([P, 1024], f32, tag="big", name="big")

            ones16 = cp.tile([P, P], f16, tag="ones16")
            nc.vector.memset(ones16[:], 1.0)
            onesrow = cp.tile([1, LT], f16, tag="onesrow")
            nc.vector.memset(onesrow[:], 1.0)
            expb = cp.tile([P, 1], f32, tag="expb")
            nc.vector.memset(expb[:], EXP_BIAS)

            # ---- resident small inputs ----
            condt = cp.tile([P, KC], f16, tag="condt")
            nc.sync.dma_start(condt[:], cond16[:])
            lnt = cp.tile([P, 4, KC], f32, tag="lnt")
            nc.sync.dma_start(lnt[:], lnf.rearrange("p (w c) -> p w c", c=KC))
            modbt = cp.tile([P, 6, KC], f32, tag="modbt")
            nc.sync.dma_start(modbt[:], modbf.rearrange("p (w c) -> p w c", c=KC))
            qkvbt = cp.tile([P, 16], f32, tag="qkvbt")
            nc.sync.dma_start(qkvbt[:], qkvbf[:])
            fc1bt = cp.tile([P, 32], f32, tag="fc1bt")
            nc.sync.dma_start(fc1bt[:], fc1bf[:])
            bvt = cp.tile([1, D], f16, tag="bvt")
            nc.sync.dma_start(bvt[:], bvrow[:])
            pbrow = cp.tile([1, D], f16, tag="pbrow")
            nc.sync.dma_start(pbrow[:], projbrow[:])
            f2brow = cp.tile([1, D], f16, tag="f2brow")
            nc.sync.dma_start(f2brow[:], fc2brow[:])

            # x, feature-major, 4 MiB (tag A4 later reused by h16)
            xf = ar.tile([P, KC, NT], f32, tag="A4")
            nc.sync.dma_start(xf[:], xt.rearrange("(c p) t -> p c t", p=P))

            # ---- S1: modulation matvecs (feature-major out) ----
            modv = cp.tile([P, 6, KC], f32, tag="modv")

            def mod_matvec(w):
                pm = pp()  # [P, 8] accum lives in a [P,512] slot
                for half in range(2):
                    mwt = ar.tile([P, KC, 512], f16, tag="Q1", bufs=3)
                    nc.sync.dma_start(
                        mwt[:],
                        modw[(w * 2 + half) * P:(w * 2 + half + 1) * P, :]
                        .rearrange("p (c m) -> p c m", m=512))
                    for mt in range(4):
                        mg = half * 4 + mt
                        for kc in range(KC):
                            nc.tensor.matmul(pm[:, mg:mg + 1],
                                             mwt[:, kc, mt * P:(mt + 1) * P],
                                             condt[:, kc:kc + 1],
                                             start=(kc == 0), stop=(kc == KC - 1))
                nc.vector.tensor_tensor(modv[:, w], pm[:, 0:KC], modbt[:, w],
                                        AluOpType.add)

            vecs = cp.tile([P, 6, KC], f32, tag="vecs")
            tgp = cp.tile([P, 2, KC], f32, tag="tgp")
            for w in range(2):
                mod_matvec(w)
            # scale1, shift1 (gate LN1 apply) from w0/w1 only
            nc.vector.tensor_scalar_add(tgp[:, 0], modv[:, 0], 1.0)
            nc.vector.tensor_tensor(vecs[:, 0], tgp[:, 0], lnt[:, 0], AluOpType.mult)
            nc.vector.tensor_tensor(vecs[:, 1], tgp[:, 0], lnt[:, 1], AluOpType.mult)
            nc.vector.tensor_tensor(vecs[:, 1], vecs[:, 1], modv[:, 1], AluOpType.add)
            # qkv weight block: put its DMA ahead of the remaining 8 MiB of
            # modulation weights in the SP queue so qkv can start on time
            qkA = ar.tile([P, KC, 2 * D], f16, tag="W4")
            nc.sync.dma_start(qkA[:],
                              qkvw[:, 0:2 * D].rearrange("(c p) m -> p c m", p=P))
            mod_matvec(2)
            nc.scalar.activation(vecs[:, 2], modv[:, 2], AF.Tanh)
            nc.vector.tensor_scalar_mul(vecs[:, 2], vecs[:, 2], GATE)

            def late_mod():
                for w in range(3, 6):
                    mod_matvec(w)
                nc.vector.tensor_scalar_add(tgp[:, 1], modv[:, 3], 1.0)
                nc.vector.tensor_tensor(vecs[:, 3], tgp[:, 1], lnt[:, 2],
                                        AluOpType.mult)
                nc.vector.tensor_tensor(vecs[:, 4], tgp[:, 1], lnt[:, 3],
                                        AluOpType.mult)
                nc.vector.tensor_tensor(vecs[:, 4], vecs[:, 4], modv[:, 4],
                                        AluOpType.add)
                nc.scalar.activation(vecs[:, 5], modv[:, 5], AF.Tanh)
                nc.vector.tensor_scalar_mul(vecs[:, 5], vecs[:, 5], GATE)

            def r32(tag="R32"):
                return rot.tile([P, NT], f32, tag=tag, bufs=4, name="r32")

            def layernorm(src, ntok, scale_col, shift_col, out16):
                halves = ntok // 512
                pss = pbig()
                psq = pbig()
                for j in range(KC):
                    c16 = rot.tile([P, NT], f16, tag="R16", bufs=3)
                    nc.scalar.activation(c16[:, 0:ntok], src[:, j], AF.Copy)
                    s16 = rot.tile([P, NT], f16, tag="R16", bufs=3)
                    nc.vector.tensor_tensor(s16[:, 0:ntok], c16[:, 0:ntok],
                                            c16[:, 0:ntok], AluOpType.mult)
                    for nh in range(halves):
                        sl = slice(nh * 512, (nh + 1) * 512)
                        nc.tensor.matmul(pss[:, sl], ones16[:], c16[:, sl],
                                         start=(j == 0), stop=(j == KC - 1),
                                         skip_group_check=True)
                        nc.tensor.matmul(psq[:, sl], ones16[:], s16[:, sl],
                                         start=(j == 0), stop=(j == KC - 1),
                                         skip_group_check=True)
                murep = r32()
                nc.vector.tensor_scalar_mul(murep[:, 0:ntok], pss[:, 0:ntok],
                                            1.0 / D)
                msq = r32()
                nc.vector.tensor_scalar(msq[:, 0:ntok], psq[:, 0:ntok],
                                        1.0 / D, EPS,
                                        AluOpType.mult, AluOpType.add)
                mu2 = r32()
                nc.vector.tensor_tensor(mu2[:, 0:ntok], murep[:, 0:ntok],
                                        murep[:, 0:ntok], AluOpType.mult)
                var = r32()
                nc.vector.tensor_tensor(var[:, 0:ntok], msq[:, 0:ntok],
                                        mu2[:, 0:ntok], AluOpType.subtract)
                rvar = r32()
                nc.vector.reciprocal(rvar[:, 0:ntok], var[:, 0:ntok])
                arep = r32()
                nc.scalar.activation(arep[:, 0:ntok], rvar[:, 0:ntok], AF.Sqrt)
                for j in range(KC):
                    t1 = r32()
                    nc.vector.tensor_tensor(t1[:, 0:ntok], src[:, j],
                                            murep[:, 0:ntok], AluOpType.subtract)
                    t2 = r32()
                    nc.vector.tensor_tensor(t2[:, 0:ntok], t1[:, 0:ntok],
                                            arep[:, 0:ntok], AluOpType.mult)
                    nc.vector.tensor_scalar(out16[:, j], t2[:, 0:ntok],
                                            vecs[:, scale_col, j:j + 1],
                                            vecs[:, shift_col, j:j + 1],
                                            AluOpType.mult, AluOpType.add)

            late_mod()

            # ---- S2/S3: LN1 + modulate (all 1024 tokens) ----
            y16 = ar.tile([P, KC, NT], f16, tag="Y2", bufs=2)
            layernorm(xf, NT, 0, 1, y16)

            # ---- S4: qkv ----
            q16 = ar.tile([P, KC, LT], f16, tag="Q1", bufs=3)
            k16 = ar.tile([P, KC, NT], f16, tag="K2")
            v16 = ar.tile([P, KC, D], f16, tag="V2")
            for mt in range(KC):  # q, local tokens
                pq = pp()
                for kc in range(KC):
                    nc.tensor.matmul(pq[:], qkA[:, kc, mt * P:(mt + 1) * P],
                                     y16[:, kc, 0:LT],
                                     start=(kc == 0), stop=(kc == KC - 1))
                nc.scalar.activation(q16[:, mt], pq[:], AF.Identity,
                                     bias=qkvbt[:, mt:mt + 1])
            for mt in range(KC):  # k, all tokens
                for nh in range(2):
                    pk = pp()
                    for kc in range(KC):
                        nc.tensor.matmul(
                            pk[:], qkA[:, kc, D + mt * P:D + (mt + 1) * P],
                            y16[:, kc, nh * 512:(nh + 1) * 512],
                            start=(kc == 0), stop=(kc == KC - 1))
                    nc.scalar.activation(k16[:, mt, nh * 512:(nh + 1) * 512],
                                         pk[:], AF.Identity,
                                         bias=qkvbt[:, 8 + mt:9 + mt])
            vW = ar.tile([P, KC, D], f16, tag="Y2", bufs=2)
            nc.sync.dma_start(vW[:],
                              qkvw[:, 2 * D:3 * D].rearrange("(c p) m -> p c m", p=P))
            pb = pbig()  # v bias replicated across partitions
            for nh in range(2):
                nc.tensor.matmul(pb[:, nh * 512:(nh + 1) * 512], ones16[0:1, :],
                                 bvt[:, nh * 512:(nh + 1) * 512],
                                 start=True, stop=True, skip_group_check=True)
            bvrep = r32()
            nc.vector.tensor_copy(bvrep[:], pb[:])
            for tt in range(KC):  # v rows = tokens (all)
                pv = pbig()
                for kc in range(KC):
                    for nh in range(2):
                        nc.tensor.matmul(
                            pv[:, nh * 512:(nh + 1) * 512],
                            y16[:, kc, tt * P:(tt + 1) * P],
                            vW[:, kc, nh * 512:(nh + 1) * 512],
                            start=(kc == 0), stop=(kc == KC - 1),
                            skip_group_check=True)
                nc.vector.tensor_tensor(v16[:, tt], pv[:], bvrep[:], AluOpType.add)

            # ---- S5: attention, head pair (2g, 2g+1) per feature tile g ----
            attn16 = ar.tile([P, KC, LT], f16, tag="AT")
            for g in range(KC):
                eg = ar.tile([P, KC, NT], f16, tag="Y2", bufs=2)
                for c in range(KC):
                    psc = pbig()
                    nc.tensor.matmul(psc[:, 0:512],
                                     k16[0:DH, g, c * P:(c + 1) * P],
                                     q16[0:DH, g, :], start=True, stop=True,
                                     skip_group_check=True)
                    nc.tensor.matmul(psc[:, 512:1024],
                                     k16[DH:P, g, c * P:(c + 1) * P],
                                     q16[DH:P, g, :], start=True, stop=True,
                                     skip_group_check=True)
                    nc.scalar.activation(eg[:, c], psc[:], AF.Exp,
                                         scale=EXP_SCALE, bias=expb[:])
                pse = pbig()
                for c in range(KC):
                    for nh in range(2):
                        nc.tensor.matmul(pse[:, nh * 512:(nh + 1) * 512],
                                         ones16[:],
                                         eg[:, c, nh * 512:(nh + 1) * 512],
                                         start=(c == 0), stop=(c == KC - 1),
                                         skip_group_check=True)
                recip = r32()
                nc.vector.reciprocal(recip[:], pse[:])
                pav = pp()
                for c in range(KC):
                    nc.tensor.matmul(pav[0:DH, :],
                                     v16[:, c, 2 * g * DH:(2 * g + 1) * DH],
                                     eg[:, c, 0:512],
                                     start=(c == 0), stop=(c == KC - 1),
                                     skip_group_check=True)
                    nc.tensor.matmul(pav[DH:P, :],
                                     v16[:, c, (2 * g + 1) * DH:(2 * g + 2) * DH],
                                     eg[:, c, 512:1024],
                                     start=(c == 0), stop=(c == KC - 1),
                                     skip_group_check=True, tile_position=(0, 64))
                nc.vector.tensor_tensor(attn16[0:DH, g], pav[0:DH, :],
                                        recip[0:DH, 0:512], AluOpType.mult)
                nc.vector.tensor_tensor(attn16[DH:P, g], pav[DH:P, :],
                                        recip[DH:P, 512:1024], AluOpType.mult)

            # ---- S6: proj + gated residual ----
            pw = ar.tile([P, KC, D], f16, tag="K2")
            nc.sync.dma_start(pw[:], projw.rearrange("(c p) m -> p c m", p=P))
            x2 = ar.tile([P, KC, LT], f32, tag="V2")
            for mt in range(KC):
                pj = pp()
                for kc in range(KC):
                    nc.tensor.matmul(pj[:], pw[:, kc, mt * P:(mt + 1) * P],
                                     attn16[:, kc, :],
                                     start=(kc == 0), stop=False)
                nc.tensor.matmul(pj[:], pbrow[:, mt * P:(mt + 1) * P],
                                 onesrow[:], start=False, stop=True)
                nc.vector.scalar_tensor_tensor(x2[:, mt], pj[:],
                                               vecs[:, 2, mt:mt + 1],
                                               xf[:, mt, 0:LT],
                                               AluOpType.mult, AluOpType.add)

            # ---- S7: LN2 + modulate (local tokens) ----
            z16 = ar.tile([P, KC, LT], f16, tag="Q1", bufs=3)
            layernorm(x2, LT, 3, 4, z16)

            # ---- S8: fc1 + gelu ----
            h16 = ar.tile([P, 32, LT], f16, tag="A4")
            f1a = ar.tile([P, KC, 2 * D], f16, tag="W4")
            nc.sync.dma_start(f1a[:],
                              fc1w[:, 0:2 * D].rearrange("(c p) m -> p c m", p=P))
            f1b1 = ar.tile([P, KC, D], f16, tag="K2")
            nc.sync.dma_start(f1b1[:],
                              fc1w[:, 2 * D:3 * D].rearrange("(c p) m -> p c m", p=P))

            def fc1_block(wt, mg0, nmt):
                for mt in range(nmt):
                    mg = mg0 + mt
                    ph = pp()
                    for kc in range(KC):
                        nc.tensor.matmul(ph[:], wt[:, kc, mt * P:(mt + 1) * P],
                                         z16[:, kc, :],
                                         start=(kc == 0), stop=(kc == KC - 1))
                    nc.scalar.activation(h16[:, mg], ph[:], AF.Gelu,
                                         bias=fc1bt[:, mg:mg + 1])

            fc1_block(f1a, 0, 16)
            f1b2 = ar.tile([P, KC, D], f16, tag="W4")
            nc.sync.dma_start(f1b2[:],
                              fc1w[:, 3 * D:4 * D].rearrange("(c p) m -> p c m", p=P))
            fc1_block(f1b1, 16, 8)
            fc1_block(f1b2, 24, 8)

            # ---- S9: fc2 + gated residual + store ----
            for mt in range(KC):
                f2col = ar.tile([P, 32, P], f16, tag="Q1", bufs=3)
                nc.sync.dma_start(
                    f2col[:],
                    fc2w[mt * P:(mt + 1) * P, :]
                    .rearrange("p (c m) -> p c m", m=P))
                pz = pp()
                for kc in range(32):
                    nc.tensor.matmul(pz[:], f2col[:, kc, :], h16[:, kc, :],
                                     start=(kc == 0), stop=False)
                nc.tensor.matmul(pz[:], f2brow[:, mt * P:(mt + 1) * P],
                                 onesrow[:], start=False, stop=True)
                ot = rot.tile([P, LT], f32, tag="OT", bufs=2)
                nc.vector.scalar_tensor_tensor(ot[:], pz[:],
                                               vecs[:, 5, mt:mt + 1],
                                               x2[:, mt, :],
                                               AluOpType.mult, AluOpType.add)
                nc.sync.dma_start(outt[mt * P:(mt + 1) * P, :], ot[:])

    n = _legalize_waits(nc)
    return nc


_NC_CACHE = {}


def _get_nc():
    if "nc" not in _NC_CACHE:
        _NC_CACHE["nc"] = _build()
    return _NC_CACHE["nc"]


def _feat(v, cols):
    """[D*]-vector -> feature-major [128, cols] (col j = chunk j)."""
    return np.ascontiguousarray(v.reshape(cols, P).T)


def make_in_maps(x, cond, g1_w, g1_b, b1_w, b1_b, a1_w, a1_b,
                 g2_w, g2_b, b2_w, b2_b, a2_w, a2_b,
                 ln1_g, ln1_b, ln2_g, ln2_b,
                 qkv_w, qkv_b, proj_w, proj_b,
                 fc1_w, fc1_b, fc2_w, fc2_b):
    f32 = np.float32
    f16 = np.float16
    x = np.asarray(x, f32)
    cond = np.asarray(cond, f32)
    shared = {
        "qkvw": np.asarray(qkv_w, f16),
        "qkvbf": np.hstack([_feat(np.asarray(qkv_b, f32)[0:D], KC),
                            _feat(np.asarray(qkv_b, f32)[D:2 * D], KC)]),
        "bvrow": np.asarray(qkv_b, f16)[None, 2 * D:3 * D],
        "projw": np.asarray(proj_w, f16),
        "projbrow": np.asarray(proj_b, f16)[None, :],
        "fc1w": np.asarray(fc1_w, f16),
        "fc1bf": _feat(np.asarray(fc1_b, f32), 32),
        # [mt*128+p, kc*128+m] = fc2_w[kc*128+p, mt*128+m]: contiguous
        # per-mt loads of the feature-major lhsT tiles
        "fc2w": np.ascontiguousarray(
            np.asarray(fc2_w, f16).reshape(32, P, KC, P)
            .transpose(2, 1, 0, 3).reshape(D, DFF)),
        "fc2brow": np.asarray(fc2_b, f16)[None, :],
        # row (w*2+half)*128+p, col kc*512+m = W_w[kc*128+p, half*512+m]:
        # contiguous 1 MiB loads of each feature-major half-block
        "modw": np.ascontiguousarray(
            np.hstack([np.asarray(w, f16) for w in
                       (g1_w, b1_w, a1_w, g2_w, b2_w, a2_w)])
            .reshape(KC, P, 6, 2, 512).transpose(2, 3, 1, 0, 4)
            .reshape(12 * P, 4 * D)),
        "modbf": np.hstack([_feat(np.asarray(v, f32), KC) for v in
                            (g1_b, b1_b, a1_b, g2_b, b2_b, a2_b)]),
        "lnf": np.hstack([_feat(np.asarray(v, f32), KC) for v in
                          (ln1_g, ln1_b, ln2_g, ln2_b)]),
    }
    in_maps = []
    for c in range(8):
        b, h = c // 2, c % 2
        xb = x[b].T  # [D, NT]
        perm = np.concatenate([np.arange(h * LT, (h + 1) * LT),
                               np.arange((1 - h) * LT, (2 - h) * LT)])
        m = dict(shared)
        m["xt"] = np.ascontiguousarray(xb[:, perm])
        m["cond16"] = _feat(cond[b], KC).astype(f16)
        in_maps.append(m)
    return in_maps


def kernel(**inputs):
    nc = _get_nc()
    in_maps = make_in_maps(**inputs)
    res = run_bass_kernel_spmd(nc, in_maps, list(range(8)))
    out = np.empty((B, NT, D), np.float32)
    for c in range(8):
        b, h = c // 2, c % 2
        out[b, h * LT:(h + 1) * LT, :] = res.results[c]["outt"].T
    return out

